# revision 1
# baseline (speedup 1.0000x reference)
"""Trainium2 Bass kernel for nn_EnergyLoss: batched 16x16 complex Hermitian
ground-state projector via shifted matrix-squaring power iteration.

Math summary (all derived from the reference):
  H[n] = 0.5*G - 0.5*sum_d X[n,d]*S_d + (0.5*q_n + EPS)*I,
     G = sum_d A_d A_d^H,  S_d = A_d + A_d^H,  q_n = sum_d X[n,d]^2
  B0 = I - H/||H||_F  (PSD shift; ground state of H = dominant eigvec of B0)
  B <- B^2 / ||B||_F^2   (13x; converges to ground-state projector P/tr(P))
  loss terms from P via rowsums: pos[n,d] = Re(sum_j colsumA[d,j]*rowsumP[n,j])/tr
Complex 16x16 matrices are embedded as real symmetric 32x32 M(B) =
[[Br,-Bi],[Bi,Br]]; per-sample squaring runs as 32x32 PE-array tile matmuls
(4 samples per 128 partitions, diag tiles).  State is fp16, PSUM fp32.
"""

import numpy as np

N, D, DIM = 4096, 32, 16
NCORES = 8
NS = N // NCORES          # 512 samples per core
NQ = NS // 4              # 128 quads (4 samples stacked per 128 partitions)
EPS = 1e-5
LAM = 0.1
KSTEPS = 13
NSLAB = 2                 # quad slabs for pipelining
QS = NQ // NSLAB          # 64 quads per slab

_prog_cache = {}

# packed constant-input byte offsets (per partition)
OFF_XBLK = 0          # f32 [128,128]  512B
OFF_MASKB = 512       # f32 [128,128]  512B
OFF_SIGNP = 1024      # f32 [128,1]    4B
OFF_SIGNPM = 1028     # f32 [128,1]    4B
OFF_XTH = 1040        # f16 [34,512]   1024B
OFF_WH = 2064         # f16 [34,512]   1024B
OFF_WPOS = 3088       # f16 [128,128]  256B
OFF_WEA2 = 3344       # f16 [128,128]  256B
OFF_DIAGP = 3600      # f16 [128,2048] 4096B
CIN_BYTES = 7696


def _build_host_tensors(A_real, A_imag, X):
    """All small A-derived tensors + per-core X-derived layouts (numpy fp32)."""
    A = (A_real + 1j * A_imag).astype(np.complex64)
    Sc = A + np.conj(np.transpose(A, (0, 2, 1)))        # [D,16,16] Hermitian
    Sr, Si = Sc.real.astype(np.float32), Sc.imag.astype(np.float32)
    G = np.einsum('dij,dkj->ik', A, A.conj())
    Gr, Gi = G.real.astype(np.float32), G.imag.astype(np.float32)
    cA = A.sum(axis=1)                                   # [D,16] colsum over i
    cA2 = (A @ A).sum(axis=1)

    # H-build weights: WH[k, 32j+m], contraction k: 0..31 = d, 32 = const, 33 = q
    WH = np.zeros((34, 512), np.float32)
    for j in range(DIM):
        c = 32 * j
        WH[:D, c:c+16] = -0.5 * Sr[:, :, j]              # m<16 -> Hr[m,j]
        WH[:D, c+16:c+32] = -0.5 * Si[:, :, j]           # m>=16 -> Hi[m-16,j]
        WH[32, c:c+16] = 0.5 * Gr[:, j]
        WH[32, c+j] += EPS
        WH[32, c+16:c+32] = 0.5 * Gi[:, j]
        WH[33, c+j] = 0.5
    # diag delta pattern on the state layout (top halves only)
    DIAGP = np.zeros((128, 16 * NQ), np.float32)
    for s in range(4):
        for i in range(DIM):
            DIAGP[32*s + i, i::16] = 1.0
    # block mask for cross-partition per-sample sums
    MASKB = np.zeros((128, 128), np.float32)
    for b in range(4):
        MASKB[32*b:32*b+32, 32*b:32*b+32] = 1.0
    SIGNP = np.ones((128, 1), np.float32)
    for s in range(4):
        SIGNP[32*s+16:32*s+32, 0] = -1.0
    # finish functionals: rhs is RS from S2 = [Pr; -Pi] rowsums
    #   pos_raw[32s+d, q] = sum_i cAr[d,i]*rr[i] - cAi[d,i]*ri[i]
    #   RS bottom rows hold -ri  =>  bottom weight = +cAi
    WPOS = np.zeros((128, 128), np.float32)
    WEA2 = np.zeros((128, 128), np.float32)
    for s in range(4):
        b = 32 * s
        WPOS[b:b+16, b:b+32] = cA.real.T                 # [i, d]
        WPOS[b+16:b+32, b:b+32] = cA.imag.T
        WEA2[b:b+16, b:b+32] = cA2.real.T
        WEA2[b+16:b+32, b:b+32] = cA2.imag.T

    # Pack everything into one u8 [128, CIN_BYTES] tensor per core so all
    # constants arrive via ONE DMA (matmul instrs only support 1 sync wait).
    def put(buf, rows, off, arr):
        b = np.ascontiguousarray(arr).view(np.uint8).reshape(arr.shape[0], -1)
        buf[:rows, off:off+b.shape[1]] = b

    per_core = []
    for c in range(NCORES):
        Xc = np.asarray(X[c*NS:(c+1)*NS], np.float32)    # [512, 32]
        q = (Xc.astype(np.float32) ** 2).sum(1)
        XTH = np.zeros((34, 512), np.float32)
        XBLK = np.zeros((128, 128), np.float32)
        for s in range(4):
            idx = np.arange(NQ) * 4 + s                  # n_core(q,s)
            XTH[:D, 128*s:128*s+128] = Xc[idx].T
            XTH[32, 128*s:128*s+128] = 1.0
            XTH[33, 128*s:128*s+128] = q[idx]
            XBLK[32*s:32*s+32, :] = Xc[idx].T
        buf = np.zeros((128, CIN_BYTES), np.uint8)
        put(buf, 128, OFF_XBLK, XBLK)
        put(buf, 128, OFF_MASKB, MASKB)
        put(buf, 128, OFF_SIGNP, SIGNP)
        put(buf, 128, OFF_SIGNPM, -SIGNP)
        put(buf, 34, OFF_XTH, XTH.astype(np.float16))
        put(buf, 34, OFF_WH, WH.astype(np.float16))
        put(buf, 128, OFF_WPOS, WPOS.astype(np.float16))
        put(buf, 128, OFF_WEA2, WEA2.astype(np.float16))
        put(buf, 128, OFF_DIAGP, DIAGP.astype(np.float16))
        per_core.append({"cin": buf})
    return per_core


def build_program(skip_shuffle=False, skip_norm=False, skip_mm=False,
                  skip_cast=False, ksteps=KSTEPS):
    import concourse.bass as bass
    import concourse.bacc as bacc
    import concourse.mybir as mybir
    import concourse.tile as tile
    from contextlib import ExitStack

    f16, f32 = mybir.dt.float16, mybir.dt.float32
    Alu = mybir.AluOpType
    Act = mybir.ActivationFunctionType

    u8 = mybir.dt.uint8
    nc = bacc.Bacc()
    # dram I/O
    d_cin = nc.dram_tensor("cin", [128, CIN_BYTES], u8, kind="ExternalInput")
    d_out = nc.dram_tensor("out", [128, 1], f32, kind="ExternalOutput")

    with tile.TileContext(nc) as tc, ExitStack() as ctx:
        cpool = ctx.enter_context(tc.tile_pool(name="consts", bufs=1))
        spool = ctx.enter_context(tc.tile_pool(name="state", bufs=2))
        wpool = ctx.enter_context(tc.tile_pool(name="work", bufs=2))
        ppool_pm = ctx.enter_context(tc.tile_pool(name="psum_pm", bufs=3, space="PSUM"))
        ppool_sm = ctx.enter_context(tc.tile_pool(name="psum_sm", bufs=2, space="PSUM"))

        cst = cpool.tile([128, CIN_BYTES], u8, tag="cin")
        nc.sync.dma_start(cst[:, :], d_cin[:, :])
        xblk = cst[:, OFF_XBLK:OFF_XBLK+512].bitcast(f32)
        maskb = cst[:, OFF_MASKB:OFF_MASKB+512].bitcast(f32)
        signp = cst[:, OFF_SIGNP:OFF_SIGNP+4].bitcast(f32)
        signpm = cst[:, OFF_SIGNPM:OFF_SIGNPM+4].bitcast(f32)
        xth = cst[:, OFF_XTH:OFF_XTH+1024].bitcast(f16)[0:34, :]
        wh = cst[:, OFF_WH:OFF_WH+1024].bitcast(f16)[0:34, :]
        wpos = cst[:, OFF_WPOS:OFF_WPOS+256].bitcast(f16)
        wea2 = cst[:, OFF_WEA2:OFF_WEA2+256].bitcast(f16)
        diagp = cst[:, OFF_DIAGP:OFF_DIAGP+4096].bitcast(f16)

        # ---------------- phase 1+2: H build, then B0 = I - H/fro ----------
        # Two PSUM halves (j in [0,8) and [8,16)); col = 128*jj + q.
        JH = DIM // 2
        pmh = []
        for h in range(2):
            ph = ppool_pm.tile([128, JH * 128], f32, tag="pm")
            for jj in range(JH):
                j = h * JH + jj
                for s in range(4):
                    nc.tensor.matmul(
                        ph[32*s:32*s+32, 128*jj:128*jj+128],
                        wh[:, 32*j:32*j+32],
                        xth[:, 128*s:128*s+128],
                        start=True, stop=True,
                        tile_position=(0, 32*s),
                    )
            pmh.append(ph)
        # fro^2 = per-sample sum of squares of H entries
        prh = wpool.tile([128, 128], f32, tag="pr")
        for h in range(2):
            sqh = wpool.tile([128, JH * 128], f32, tag="sqh")
            nc.scalar.activation(sqh[:, :], pmh[h][:, :], Act.Square)
            if h == 0:
                nc.vector.tensor_reduce(
                    prh[:, :], sqh[:, :].rearrange("p (j q) -> p q j", j=JH),
                    axis=mybir.AxisListType.X, op=Alu.add)
            else:
                prh2 = wpool.tile([128, 128], f32, tag="pr2")
                nc.vector.tensor_reduce(
                    prh2[:, :], sqh[:, :].rearrange("p (j q) -> p q j", j=JH),
                    axis=mybir.AxisListType.X, op=Alu.add)
                nc.vector.tensor_tensor(prh[:, :], prh[:, :], prh2[:, :],
                                        op=Alu.add)
        trh = ppool_sm.tile([128, 128], f32, tag="sm")
        nc.tensor.matmul(trh[:, :], maskb[:, :], prh[:, :], start=True, stop=True)
        rcph = wpool.tile([128, 128], f32, tag="scl")
        nc.vector.reciprocal(rcph[:, :], trh[:, :])          # 1/fro^2
        invf = wpool.tile([128, 128], f32, tag="scl2")
        nc.scalar.activation(invf[:, :], rcph[:, :], Act.Sqrt)  # 1/fro
        scl2h = wpool.tile([128, 128], f32, tag="scl3")
        nc.vector.tensor_scalar_mul(scl2h[:, :], invf[:, :], signpm[:, :])

        s2 = spool.tile([128, 2048], f16, tag="s2")          # [Br; -Bi] dense
        # s2 = PMH * (-sign*invf)  (reordered (j,q) -> (q,j)) then += diag
        for h in range(2):
            nc.vector.tensor_tensor(
                s2[:, :].rearrange("p (q j) -> p q j", j=DIM)[:, :, h*JH:(h+1)*JH],
                pmh[h][:, :].rearrange("p (j q) -> p q j", j=JH),
                scl2h[:, :].unsqueeze(-1).broadcast_to([128, 128, JH]),
                op=Alu.mult)
        nc.vector.tensor_tensor(s2[:, :], s2[:, :], diagp[:, :], op=Alu.add)

        HSWAP = list(range(16, 32)) + list(range(0, 16))
        u32 = mybir.dt.uint32

        def build_wb(wb_t, s2_t, sl):
            """wb[:, 32q+0:16] = s2*signp (-> [Br;Bi]);
            wb[:, 32q+16:32] = partition-half-swapped s2 (-> [-Bi;Br])."""
            c0, c1 = 16 * sl * QS, 16 * (sl + 1) * QS
            wbl = wb_t[:, :].rearrange("p (q j) -> p q j", j=32)
            nc.scalar.activation(
                wbl[:, sl*QS:(sl+1)*QS, 0:16],
                s2_t[:, c0:c1].rearrange("p (q j) -> p q j", j=DIM),
                Act.Copy, scale=signp[:, :])
            if skip_shuffle:
                return
            # swap via u32 view (halves the element count)
            wbw = wb_t[:, :].bitcast(u32).rearrange("p (q w) -> p q w", w=16)
            s2w = s2_t[:, :].bitcast(u32)
            nc.vector.stream_shuffle(
                wbw[:, sl*QS:(sl+1)*QS, 8:16],
                s2w[:, 8*sl*QS:8*(sl+1)*QS].rearrange("p (q w) -> p q w", w=8),
                mask=HSWAP)

        wb = spool.tile([128, 4096], f16, tag="wb")
        for sl in range(NSLAB):
            build_wb(wb, s2, sl)

        # ---------------- phase 3: iteration ----------------
        for k in range(ksteps):
            last = (k == ksteps - 1)
            s2n = spool.tile([128, 2048], f16, tag="s2")
            wbn = None if last else spool.tile([128, 4096], f16, tag="wb")
            exact = (k % 2 == 0) and not skip_norm
            for sl in range(NSLAB):
                q0 = sl * QS
                if exact:
                    # normalizer from input state (tr(B^2) = ||B||_F^2)
                    sq = wpool.tile([128, 16*QS], f16, tag=f"sq{sl}")
                    nc.scalar.activation(sq[:, :], s2[:, 16*q0:16*(q0+QS)],
                                         Act.Square)
                    pr = wpool.tile([128, QS], f32, tag=f"pr{sl}")
                    nc.vector.tensor_reduce(
                        pr[:, :], sq[:, :].rearrange("p (q j) -> p q j", j=DIM),
                        axis=mybir.AxisListType.X, op=Alu.add)
                    trp = ppool_sm.tile([128, QS], f32, tag="sm")
                    nc.tensor.matmul(trp[:, :], maskb[:, :], pr[:, :],
                                     start=True, stop=True)
                    scl = wpool.tile([128, QS], f32, tag=f"scl{sl}")
                    nc.vector.reciprocal(scl[:, :], trp[:, :])
                    scl2 = wpool.tile([128, QS], f32, tag=f"scl2{sl}")
                    nc.vector.tensor_scalar_mul(scl2[:, :], scl[:, :], signp[:, :])

                # squaring matmuls: per quad 4 diagonal 32x32-tile MMs
                pm = ppool_pm.tile([128, 16*QS], f32, tag="pm")
                mmr = range(0 if not skip_mm else QS - 1, QS)
                for qq in mmr:
                    q = q0 + qq
                    for s in range(4):
                        nc.tensor.matmul(
                            pm[32*s:32*s+32, 16*qq:16*qq+16],
                            wb[32*s:32*s+32, 32*q:32*q+32],
                            wb[32*s:32*s+32, 32*q:32*q+16],
                            start=True, stop=True,
                            tile_position=(32*s, 32*s))
                if skip_cast:
                    nc.scalar.activation(
                        s2n[:, 16*q0:16*q0+16], pm[:, 0:16], Act.Copy)
                    if not last:
                        build_wb(wbn, s2n, sl)
                    continue
                if exact:
                    # cast: s2' = pm * (sign/fro2)
                    nc.vector.tensor_tensor(
                        s2n[:, 16*q0:16*(q0+QS)].rearrange("p (q j) -> p q j",
                                                           j=DIM),
                        pm[:, :].rearrange("p (q j) -> p q j", j=DIM),
                        scl2[:, :].unsqueeze(-1).broadcast_to([128, QS, DIM]),
                        op=Alu.mult)
                else:
                    # cast: s2' = pm * sign (no normalization this step)
                    nc.scalar.activation(
                        s2n[:, 16*q0:16*(q0+QS)].rearrange("p (q j) -> p q j",
                                                           j=DIM),
                        pm[:, :].rearrange("p (q j) -> p q j", j=DIM),
                        Act.Copy, scale=signp[:, :])
                if not last:
                    build_wb(wbn, s2n, sl)
            s2 = s2n
            if not last:
                wb = wbn

        # ---------------- phase 4: finish ----------------
        # rowsums of [Pr; -Pi]
        rs = wpool.tile([128, 128], f32, tag="rs")
        nc.vector.tensor_reduce(
            rs[:, :], s2[:, :].rearrange("p (q j) -> p q j", j=DIM),
            axis=mybir.AxisListType.X, op=Alu.add)
        rs16 = wpool.tile([128, 128], f16, tag="rs16")
        nc.vector.tensor_copy(rs16[:, :], rs[:, :])
        # trace of P (first, to bound live small-PSUM tiles at 2)
        trm = wpool.tile([128, 2048], f16, tag="trm")
        nc.vector.tensor_tensor(trm[:, :], s2[:, :], diagp[:, :], op=Alu.mult)
        prt = wpool.tile([128, 128], f32, tag="prt")
        nc.vector.tensor_reduce(
            prt[:, :], trm[:, :].rearrange("p (q j) -> p q j", j=DIM),
            axis=mybir.AxisListType.X, op=Alu.add)
        trf = ppool_sm.tile([128, 128], f32, tag="sm")
        nc.tensor.matmul(trf[:, :], maskb[:, :], prt[:, :], start=True, stop=True)
        invt = wpool.tile([128, 128], f32, tag="invt")
        nc.vector.reciprocal(invt[:, :], trf[:, :])

        pos = ppool_sm.tile([128, 128], f32, tag="sm")
        nc.tensor.matmul(pos[:, :], wpos[:, :], rs16[:, :], start=True, stop=True)
        posn = wpool.tile([128, 128], f32, tag="posn")
        nc.vector.tensor_tensor(posn[:, :], pos[:, :], invt[:, :], op=Alu.mult)
        ea2 = ppool_sm.tile([128, 128], f32, tag="sm")
        nc.tensor.matmul(ea2[:, :], wea2[:, :], rs16[:, :], start=True, stop=True)
        ea2n = wpool.tile([128, 128], f32, tag="ea2n")
        nc.vector.tensor_tensor(ea2n[:, :], ea2[:, :], invt[:, :], op=Alu.mult)
        terr = wpool.tile([128, 128], f32, tag="terr")
        nc.vector.tensor_tensor(terr[:, :], posn[:, :], xblk[:, :], op=Alu.subtract)
        t2 = wpool.tile([128, 128], f32, tag="t2")
        nc.vector.tensor_tensor(t2[:, :], terr[:, :], terr[:, :], op=Alu.mult)
        p2 = wpool.tile([128, 128], f32, tag="p2")
        nc.vector.tensor_tensor(p2[:, :], posn[:, :], posn[:, :], op=Alu.mult)
        vterm = wpool.tile([128, 128], f32, tag="vterm")
        nc.vector.tensor_tensor(vterm[:, :], ea2n[:, :], p2[:, :], op=Alu.subtract)
        vs = wpool.tile([128, 128], f32, tag="vs")
        nc.vector.tensor_scalar_mul(vs[:, :], vterm[:, :], LAM)
        r = wpool.tile([128, 128], f32, tag="r")
        nc.vector.tensor_tensor(r[:, :], t2[:, :], vs[:, :], op=Alu.add)
        outv = wpool.tile([128, 1], f32, tag="outv")
        nc.vector.tensor_reduce(outv[:, :], r[:, :], axis=mybir.AxisListType.X,
                                op=Alu.add)
        nc.sync.dma_start(d_out[:, :], outv[:, :])
    nc.compile()
    return nc


def kernel(A_real, A_imag, X):
    from concourse.bass_utils import run_bass_kernel_spmd

    per_core = _build_host_tensors(
        np.asarray(A_real, np.float32), np.asarray(A_imag, np.float32),
        np.asarray(X, np.float32))

    if "nc" not in _prog_cache:
        _prog_cache["nc"] = build_program()
    nc = _prog_cache["nc"]

    in_maps = [per_core[c] for c in range(NCORES)]
    res = run_bass_kernel_spmd(nc, in_maps, list(range(NCORES)))
    total = 0.0
    for c in range(NCORES):
        total += float(np.asarray(res.results[c]["out"], np.float64).sum())
    loss = total / N
    return np.float32(loss)



# revision 24
# speedup vs baseline: 1.5197x; 1.5197x over previous
"""Trainium2 Bass kernel for nn_EnergyLoss: batched 16x16 complex Hermitian
ground-state projector via shifted matrix-squaring power iteration.

Math (derived from the reference):
  H[n] = 0.5*G - 0.5*sum_d X[n,d]*S_d + (0.5*q_n + EPS)*I,
     G = sum_d A_d A_d^H,  S_d = A_d + A_d^H,  q_n = sum_d X[n,d]^2
  B0 = PF*(I - H/||H||_F)   (PSD shift, prefolded by PF=1/3 so ||B0||_F ~ 1)
  B <- B^2, renormalized by 1/||B||_F^2 on steps {2,5,8}   (12 steps total)
  B converges to c*P (ground-state projector); loss terms from P via rowsums.

Implementation notes:
  - complex 16x16 embedded as real 32x32 M(B) = [[Br,-Bi],[Bi,Br]]; 4 samples
    stacked per 128 partitions; per-sample squaring = one 32x32 PE-tile matmul
    with 16-col moving operand ([Br;Bi] half of M).
  - state per step: wb [128, 32*NQ] f16 holds full M; left 16 cols/quad are
    the t-form [Br;Bi] (cast directly from PSUM), right 16 cols [-Bi;Br] are
    stream-shuffled from u = t*signp.
  - ||H||_F^2 computed as y^T G34 y (G34 precomputed host-side, 34-dim
    y=(x,1,q)); fro itself feeds back as a 35th contraction row so the H-build
    matmul emits PF*(fro*I - H) directly (no separate diagonal add).
  - 4-slab pipelining keeps PE (the bottleneck) continuously fed; elementwise
    work is split across Act/DVE/Pool so each stays under the PE step time.
"""

import numpy as np

N, D, DIM = 4096, 32, 16
NCORES = 8
NS = N // NCORES          # 512 samples per core
NQ = NS // 4              # 128 quads (4 samples per 128 partitions)
EPS = 1e-5
LAM = 0.1
PF = 1.0 / 3.0            # prefold of B0
KSTEPS = 12
NORM_STEPS = (2, 5, 8)
NSLAB = 4
QS = NQ // NSLAB          # 32 quads per slab
GSH = 8                   # G34 scaled by 2^-GSH to keep f16 products in range

_prog_cache = {}

# ---- cinA packed layout (bytes per partition) -----------------------------
A_XTH = 0                 # f16 [35, 512]   1024B  (row 34 device-written fro)
A_WH = 1024               # f16 [35, 512]   1024B
A_G34 = 2048              # f16 [34, 64]    128B   (34 used, padded)
A_ONES = 2176             # f16 [34, 128]   256B
A_SIGNP = 2432            # f32 [128, 1]    4B (pad 16)
A_MASKB = 2448            # f32 [128, 128]  512B
CA = 2960
# ---- cinB ------------------------------------------------------------------
B_DIAGP = 0               # f16 [128, 2048] 4096B
B_WPOS = 4096             # f32 [128, 128]  512B
B_WEA2 = 4608             # f32 [128, 128]  512B
B_XBLK = 5120             # f32 [128, 128]  512B
CB = 5632


def _build_host_tensors(A_real, A_imag, X):
    A = (A_real + 1j * A_imag).astype(np.complex64)
    Sc = A + np.conj(np.transpose(A, (0, 2, 1)))        # [D,16,16] Hermitian
    Sr, Si = Sc.real.astype(np.float64), Sc.imag.astype(np.float64)
    G = np.einsum('dij,dkj->ik', A, A.conj())
    Gr, Gi = G.real.astype(np.float64), G.imag.astype(np.float64)
    cA = A.sum(axis=1)                                   # [D,16] colsum over i
    cA2 = (A @ A).sum(axis=1)

    # WH[k, 32j+m]: contraction k: 0 = fro (device-written), 1..32 = d,
    # 33 = const, 34 = q.  Emits pm = PF*(fro*I - H) in t-layout.
    WH = np.zeros((35, 512), np.float64)
    for j in range(DIM):
        c = 32 * j
        WH[0, c+j] = 1.0                                 # fro * I
        WH[1:1+D, c:c+16] = 0.5 * Sr[:, :, j]            # -(-0.5 Sr) = +0.5
        WH[1:1+D, c+16:c+32] = 0.5 * Si[:, :, j]
        WH[33, c:c+16] = -0.5 * Gr[:, j]
        WH[33, c+j] -= EPS
        WH[33, c+16:c+32] = -0.5 * Gi[:, j]
        WH[34, c+j] = -0.5
    WH *= PF

    # G35 for fro^2 = y^T G y (y = rows 1..34 of xth: (x, 1, q)); row/col 0
    # (the fro row) is zero so the runtime fro value never contributes.
    Wraw = WH[1:35].reshape(34, 16, 32) / PF
    G34 = np.einsum('kjm,ljm->kl', Wraw, Wraw) * (2.0 ** -GSH)
    G35 = np.zeros((35, 35))
    G35[1:35, 1:35] = G34

    MASKB = np.zeros((128, 128), np.float32)
    for b in range(4):
        MASKB[32*b:32*b+32, 32*b:32*b+32] = 1.0
    SIGNP = np.ones((128, 1), np.float32)
    for s in range(4):
        SIGNP[32*s+16:32*s+32, 0] = -1.0
    DIAGP = np.zeros((128, 16 * NQ), np.float32)
    for s in range(4):
        for i in range(DIM):
            DIAGP[32*s + i, i::16] = 1.0
    # finish functionals: rs is rowsums of t-form [Pr; +Pi]
    #   pos_raw[32s+d, q] = sum_i cAr[d,i]*rr[i] - cAi[d,i]*ri[i]
    WPOS = np.zeros((128, 128), np.float32)
    WEA2 = np.zeros((128, 128), np.float32)
    for s in range(4):
        b = 32 * s
        WPOS[b:b+16, b:b+32] = cA.real.T
        WPOS[b+16:b+32, b:b+32] = -cA.imag.T
        WEA2[b:b+16, b:b+32] = cA2.real.T
        WEA2[b+16:b+32, b:b+32] = -cA2.imag.T
    ONES34 = np.ones((35, 128), np.float32)

    def put(buf, off, arr, dt):
        a = np.ascontiguousarray(arr.astype(dt))
        b = a.view(np.uint8).reshape(a.shape[0], -1)
        buf[:a.shape[0], off:off+b.shape[1]] = b

    per_core = []
    for c in range(NCORES):
        Xc = np.asarray(X[c*NS:(c+1)*NS], np.float64)    # [512, 32]
        q = (Xc ** 2).sum(1)
        XTH = np.zeros((35, 512), np.float64)
        XBLK = np.zeros((128, 128), np.float32)
        for s in range(4):
            idx = np.arange(NQ) * 4 + s                  # sample (q, s)
            XTH[1:1+D, 128*s:128*s+128] = Xc[idx].T
            XTH[33, 128*s:128*s+128] = 1.0
            XTH[34, 128*s:128*s+128] = q[idx]
            XBLK[32*s:32*s+32, :] = Xc[idx].T.astype(np.float32)
        bufA = np.zeros((128, CA), np.uint8)
        put(bufA, A_XTH, XTH, np.float16)
        put(bufA, A_WH, WH, np.float16)
        g = np.zeros((35, 64), np.float64)
        g[:, :35] = G35
        put(bufA, A_G34, g, np.float16)
        put(bufA, A_ONES, ONES34, np.float16)
        put(bufA, A_SIGNP, SIGNP, np.float32)
        put(bufA, A_MASKB, MASKB, np.float32)
        bufB = np.zeros((128, CB), np.uint8)
        put(bufB, B_DIAGP, DIAGP, np.float16)
        put(bufB, B_WPOS, WPOS, np.float32)
        put(bufB, B_WEA2, WEA2, np.float32)
        put(bufB, B_XBLK, XBLK, np.float32)
        per_core.append({"cina": bufA, "cinb": bufB})
    return per_core


def build_program(ksteps=KSTEPS, norm_steps=NORM_STEPS, debug=False):
    import concourse.bass as bass
    import concourse.bacc as bacc
    import concourse.mybir as mybir
    import concourse.tile as tile
    from contextlib import ExitStack

    f16, f32 = mybir.dt.float16, mybir.dt.float32
    u8, u32 = mybir.dt.uint8, mybir.dt.uint32
    Alu = mybir.AluOpType
    Act = mybir.ActivationFunctionType
    X_AX = mybir.AxisListType.X
    HSWAP = list(range(16, 32)) + list(range(0, 16))
    norm_set = set(norm_steps)

    nc = bacc.Bacc()
    d_cina = nc.dram_tensor("cina", [128, CA], u8, kind="ExternalInput")
    d_cinb = nc.dram_tensor("cinb", [128, CB], u8, kind="ExternalInput")
    d_out = nc.dram_tensor("out", [128, 1], f32, kind="ExternalOutput")
    if debug:
        d_dbg_fro = nc.dram_tensor("dbg_fro", [128, 512], f32,
                                   kind="ExternalOutput")
        d_dbg_invf = nc.dram_tensor("dbg_invf", [128, 128], f32,
                                    kind="ExternalOutput")
        d_dbg_wb0 = nc.dram_tensor("dbg_wb0", [128, 4096], f16,
                                   kind="ExternalOutput")
        d_dbg_wb1 = nc.dram_tensor("dbg_wb1", [128, 4096], f16,
                                   kind="ExternalOutput")
        d_dbg_rs = nc.dram_tensor("dbg_rs", [128, 128], f32,
                                  kind="ExternalOutput")
        d_dbg_prt = nc.dram_tensor("dbg_prt", [128, 128], f32,
                                   kind="ExternalOutput")
        d_dbg_wb2 = nc.dram_tensor("dbg_wb2", [128, 4096], f16,
                                   kind="ExternalOutput")
        d_dbg_wb3 = nc.dram_tensor("dbg_wb3", [128, 4096], f16,
                                   kind="ExternalOutput")
        d_dbg_scl = nc.dram_tensor("dbg_scl", [128, 128], f32,
                                   kind="ExternalOutput")
        d_dbg_wbs = {
            k: nc.dram_tensor(f"dbg_wbs{k}", [128, 4096], f16,
                              kind="ExternalOutput")
            for k in range(3, KSTEPS)
        }

    with tile.TileContext(nc) as tc, ExitStack() as ctx:
        cpool = ctx.enter_context(tc.tile_pool(name="consts", bufs=1))
        spool = ctx.enter_context(tc.tile_pool(name="state", bufs=2))
        wpool = ctx.enter_context(tc.tile_pool(name="work", bufs=2))
        upool = ctx.enter_context(tc.tile_pool(name="uslab", bufs=3))
        qpool = ctx.enter_context(tc.tile_pool(name="small", bufs=2))
        pmpool = ctx.enter_context(tc.tile_pool(name="psum_pm", bufs=3,
                                                space="PSUM"))
        p1ctx = ExitStack()
        p1pool = p1ctx.enter_context(tc.tile_pool(name="psum_p1", bufs=1,
                                                  space="PSUM"))

        cina = cpool.tile([128, CA], u8, tag="cina")
        cinb = cpool.tile([128, CB], u8, tag="cinb")
        nc.sync.dma_start(cina[:, :], d_cina[:, :])
        nc.sync.dma_start(cinb[:, :], d_cinb[:, :])

        xth = cina[:, A_XTH:A_XTH+1024].bitcast(f16)[0:35, :]
        wh = cina[:, A_WH:A_WH+1024].bitcast(f16)[0:35, :]
        g34 = cina[:, A_G34:A_G34+128].bitcast(f16)[0:35, :]
        ones34 = cina[:, A_ONES:A_ONES+256].bitcast(f16)[0:35, :]
        signp = cina[:, A_SIGNP:A_SIGNP+4].bitcast(f32)
        maskb = cina[:, A_MASKB:A_MASKB+512].bitcast(f32)
        diagp = cinb[:, B_DIAGP:B_DIAGP+4096].bitcast(f16)
        wpos = cinb[:, B_WPOS:B_WPOS+512].bitcast(f32)
        wea2 = cinb[:, B_WEA2:B_WEA2+512].bitcast(f32)
        xblk = cinb[:, B_XBLK:B_XBLK+512].bitcast(f32)

        # ---------------- warmup: keep PE busy from t=0 --------------------
        wz = wpool.tile([128, 512], f16, tag="wz")
        nc.gpsimd.memset(wz[:, :], 0.0)
        pwarm = p1pool.tile([128, 512], f32, tag="warm")
        for _ in range(3):
            nc.tensor.matmul(pwarm[:, :], wz[0:32, 0:128], wz[0:32, :],
                             start=True, stop=True)

        # ---------------- fro chain: fro^2 = y^T G34 y ---------------------
        gy = p1pool.tile([35, 512], f32, tag="gy")
        nc.tensor.matmul(gy[:, :], g34[:, 0:35], xth[:, :],
                         start=True, stop=True)
        prod = wpool.tile([35, 512], f16, tag="prod")
        nc.vector.tensor_tensor(prod[:, :], gy[:, :], xth[:, :],
                                op=Alu.mult)
        fro2b = p1pool.tile([128, 512], f32, tag="fro2b")
        nc.tensor.matmul(fro2b[:, :], ones34[:, :], prod[:, :],
                         start=True, stop=True)
        # xth row 0 <- fro = sqrt(fro2b * 2^GSH)
        nc.scalar.activation(xth[0:1, :], fro2b[0:1, :], Act.Sqrt,
                             scale=float(2.0 ** GSH))
        # invf[p, q] = 1/fro of sample (q, s(p))
        invsq = wpool.tile([128, 128], f32, tag="invsq")
        for s in range(4):
            nc.vector.reciprocal(invsq[32*s:32*s+32, :],
                                 fro2b[32*s:32*s+32, 128*s:128*s+128])
        invf = wpool.tile([128, 128], f32, tag="invf")
        nc.scalar.activation(invf[:, :], invsq[:, :], Act.Sqrt,
                             scale=float(2.0 ** -GSH))

        # ---------------- helpers -----------------------------------------
        def wb_left(wb_t, sl):
            return wb_t[:, :].rearrange("p (q c) -> p q c", c=32)[
                :, sl*QS:(sl+1)*QS, 0:16]

        def wb_right_u32(wb_t, sl):
            return wb_t[:, :].bitcast(u32).rearrange("p (q w) -> p q w", w=16)[
                :, sl*QS:(sl+1)*QS, 8:16]

        def emit_trio(wbn, pm_t, sl, scl=None, cast_engine="act",
                      last=False):
            """pm [128, 16*QS] -> wbn left (t-form), u, wbn right."""
            dst = wb_left(wbn, sl)
            src = pm_t[:, :].rearrange("p (q j) -> p q j", j=16)
            if scl is not None:
                nc.vector.tensor_tensor(
                    dst, src,
                    scl.unsqueeze(-1).broadcast_to([128, QS, 16]),
                    op=Alu.mult)
            elif cast_engine == "act":
                nc.scalar.activation(dst, src, Act.Copy)
            else:
                nc.vector.tensor_copy(dst, src)
            if last:
                return
            us = upool.tile([128, 16*QS], f16, tag=f"u{sl % 2}")
            if cast_engine == "dve_u_pool":
                nc.gpsimd.tensor_scalar_mul(
                    us[:, :].rearrange("p (q j) -> p q j", j=16),
                    wb_left(wbn, sl), signp[:, :])
            else:
                nc.vector.tensor_scalar_mul(
                    us[:, :].rearrange("p (q j) -> p q j", j=16),
                    wb_left(wbn, sl), signp[:, :])
            nc.vector.stream_shuffle(
                wb_right_u32(wbn, sl),
                us[:, :].bitcast(u32).rearrange("p (q w) -> p q w", w=8),
                mask=HSWAP)

        def emit_norm_prep(wb_t, sl, sq_t, pr_t, scl_t):
            """fro^2 of state per sample -> scl_t[:, sl*QS:...] (1/fro^2)."""
            nc.gpsimd.tensor_tensor(
                sq_t[:, :].rearrange("p (q j) -> p q j", j=16),
                wb_left(wb_t, sl), wb_left(wb_t, sl), op=Alu.mult)
            nc.vector.tensor_reduce(
                pr_t[:, :], sq_t[:, :].rearrange("p (q j) -> p q j", j=16),
                axis=X_AX, op=Alu.add)
            trp = smpool.tile([128, QS], f32, tag="trp")  # noqa: F821
            nc.tensor.matmul(trp[:, :], maskb[:, :], pr_t[:, :],
                             start=True, stop=True)
            nc.vector.reciprocal(scl_t[:, sl*QS:(sl+1)*QS], trp[:, :])

        # ---------------- phase 1: H build -> B0 ---------------------------
        wb = spool.tile([128, 32*NQ], f16, tag="wb")
        for sl in range(NSLAB):
            ph = pmpool.tile([128, 16*QS], f32, tag="pm")
            for j in range(DIM):
                for s in range(4):
                    nc.tensor.matmul(
                        ph[32*s:32*s+32, 32*j:32*j+32],
                        wh[:, 32*j:32*j+32],
                        xth[:, 128*s+QS*sl:128*s+QS*sl+QS],
                        start=True, stop=True, tile_position=(0, 32*s))
            # cast1 with per-quad 1/fro (DVE), u on Act, shuffle DVE
            dst = wb_left(wb, sl)
            nc.vector.tensor_tensor(
                dst, ph[:, :].rearrange("p (j q) -> p q j", j=16),
                invf[:, QS*sl:QS*(sl+1)].unsqueeze(-1)
                    .broadcast_to([128, QS, 16]),
                op=Alu.mult)
            us = upool.tile([128, 16*QS], f16, tag=f"u{sl % 2}")
            nc.scalar.activation(us[:, :], wb_left(wb, sl), Act.Copy,
                                 scale=signp[:, :])
            nc.vector.stream_shuffle(
                wb_right_u32(wb, sl),
                us[:, :].bitcast(u32).rearrange("p (q w) -> p q w", w=8),
                mask=HSWAP)

        if debug:
            frocp = wpool.tile([128, 512], f32, tag="frocp")
            nc.vector.tensor_copy(frocp[:, :], fro2b[:, :])
            nc.sync.dma_start(d_dbg_fro[:, :], frocp[:, :])
            nc.sync.dma_start(d_dbg_invf[:, :], invf[:, :])
            nc.sync.dma_start(d_dbg_wb0[:, :], wb[:, :])

        # ---------------- iteration ----------------------------------------
        p1ctx.close()
        smpool = ctx.enter_context(tc.tile_pool(name="psum_sm", bufs=2,
                                                space="PSUM"))
        scl_t = None
        sq_t = None
        for k in range(ksteps):
            last = (k == ksteps - 1)
            is_norm = k in norm_set
            prep_next = (k + 1) in norm_set
            wbn = spool.tile([128, 32*NQ], f16, tag="wb")
            if prep_next:
                scl_next = qpool.tile([128, NQ], f32, tag="scl")
                sq_next = qpool.tile([128, 16*QS], f32, tag="sq")
            for sl in range(NSLAB):
                pm = pmpool.tile([128, 16*QS], f32, tag="pm")
                for qq in range(QS):
                    q = sl * QS + qq
                    for s in range(4):
                        nc.tensor.matmul(
                            pm[32*s:32*s+32, 16*qq:16*qq+16],
                            wb[32*s:32*s+32, 32*q:32*q+32],
                            wb[32*s:32*s+32, 32*q:32*q+16],
                            start=True, stop=True,
                            tile_position=(32*s, 32*s))
                if is_norm:
                    emit_trio(wbn, pm, sl,
                              scl=scl_t[:, sl*QS:(sl+1)*QS],
                              cast_engine="dve_u_pool", last=last)
                else:
                    emit_trio(wbn, pm, sl, cast_engine="act", last=last)
                if prep_next:
                    # fro^2 of state_{k+1} (slab just written) for step k+1
                    pr = qpool.tile([128, QS], f32, tag=f"pr{sl % 2}")
                    emit_norm_prep(wbn, sl, sq_next, pr, scl_next)
            wb = wbn
            if debug and k == 0:
                nc.sync.dma_start(d_dbg_wb1[:, :], wb[:, :])
            if debug and k == 1:
                nc.sync.dma_start(d_dbg_wb2[:, :], wb[:, :])
            if debug and k == 2:
                nc.sync.dma_start(d_dbg_wb3[:, :], wb[:, :])
            if debug and k >= 3:
                nc.sync.dma_start(d_dbg_wbs[k][:, :], wb[:, :])
            if prep_next:
                scl_t = scl_next
                sq_t = sq_next
                if debug and k == 1:
                    sclcp = wpool.tile([128, NQ], f32, tag="sclcp")
                    nc.vector.tensor_copy(sclcp[:, :], scl_t[:, :])
                    nc.sync.dma_start(d_dbg_scl[:, :], sclcp[:, :])

        # ---------------- finish -------------------------------------------
        # rowsums of t-form state (P up to scale) + trace
        rs = wpool.tile([128, NQ], f32, tag="rs")
        prt = wpool.tile([128, NQ], f32, tag="prt")
        for sl in range(NSLAB):
            nc.vector.tensor_reduce(
                rs[:, sl*QS:(sl+1)*QS],
                wb_left(wb, sl), axis=X_AX, op=Alu.add)
            trm = upool.tile([128, 16*QS], f16, tag=f"trm{sl % 2}")
            nc.gpsimd.tensor_tensor(
                trm[:, :].rearrange("p (q j) -> p q j", j=16),
                wb_left(wb, sl),
                diagp[:, :].rearrange("p (q j) -> p q j", j=16)[
                    :, sl*QS:(sl+1)*QS, :],
                op=Alu.mult)
            nc.vector.tensor_reduce(
                prt[:, sl*QS:(sl+1)*QS],
                trm[:, :].rearrange("p (q j) -> p q j", j=16),
                axis=X_AX, op=Alu.add)

        if debug:
            nc.sync.dma_start(d_dbg_rs[:, :], rs[:, :])
            nc.sync.dma_start(d_dbg_prt[:, :], prt[:, :])

        trf = smpool.tile([128, NQ], f32, tag="fin")
        nc.tensor.matmul(trf[:, :], maskb[:, :], prt[:, :], start=True,
                         stop=True)
        invt = wpool.tile([128, NQ], f32, tag="invt")
        nc.vector.reciprocal(invt[:, :], trf[:, :])
        pos = smpool.tile([128, NQ], f32, tag="fin")
        nc.tensor.matmul(pos[:, :], wpos[:, :], rs[:, :], start=True,
                         stop=True)
        ea2 = smpool.tile([128, NQ], f32, tag="fin")
        nc.tensor.matmul(ea2[:, :], wea2[:, :], rs[:, :], start=True,
                         stop=True)
        posn = wpool.tile([128, NQ], f32, tag="posn")
        nc.vector.tensor_tensor(posn[:, :], pos[:, :], invt[:, :],
                                op=Alu.mult)
        ea2n = wpool.tile([128, NQ], f32, tag="ea2n")
        nc.vector.tensor_tensor(ea2n[:, :], ea2[:, :], invt[:, :],
                                op=Alu.mult)
        terr = wpool.tile([128, NQ], f32, tag="terr")
        nc.vector.tensor_tensor(terr[:, :], posn[:, :], xblk[:, :],
                                op=Alu.subtract)
        t2 = wpool.tile([128, NQ], f32, tag="t2")
        nc.scalar.activation(t2[:, :], terr[:, :], Act.Square)
        p2 = wpool.tile([128, NQ], f32, tag="p2")
        nc.scalar.activation(p2[:, :], posn[:, :], Act.Square)
        vv = wpool.tile([128, NQ], f32, tag="vv")
        nc.vector.tensor_tensor(vv[:, :], ea2n[:, :], p2[:, :],
                                op=Alu.subtract)
        r = wpool.tile([128, NQ], f32, tag="r")
        nc.vector.scalar_tensor_tensor(r[:, :], vv[:, :], LAM, t2[:, :],
                                       op0=Alu.mult, op1=Alu.add)
        outv = wpool.tile([128, 1], f32, tag="outv")
        nc.vector.tensor_reduce(outv[:, :], r[:, :], axis=X_AX, op=Alu.add)
        nc.sync.dma_start(d_out[:, :], outv[:, :])
    nc.compile()
    return nc


def kernel(A_real, A_imag, X):
    from concourse.bass_utils import run_bass_kernel_spmd

    per_core = _build_host_tensors(
        np.asarray(A_real, np.float32), np.asarray(A_imag, np.float32),
        np.asarray(X, np.float32))

    if "nc" not in _prog_cache:
        _prog_cache["nc"] = build_program()
    nc = _prog_cache["nc"]

    res = run_bass_kernel_spmd(nc, per_core, list(range(NCORES)))
    total = 0.0
    for c in range(NCORES):
        total += float(np.asarray(res.results[c]["out"], np.float64).sum())
    return np.float32(total / N)


# revision 29
# speedup vs baseline: 1.6106x; 1.0598x over previous
"""Trainium2 Bass kernel for nn_EnergyLoss: batched 16x16 complex Hermitian
ground-state projector via shifted matrix-squaring power iteration.

Math (derived from the reference):
  H[n] = 0.5*G - 0.5*sum_d X[n,d]*S_d + (0.5*q_n + EPS)*I,
     G = sum_d A_d A_d^H,  S_d = A_d + A_d^H,  q_n = sum_d X[n,d]^2
  B0 = PF*(I - H/||H||_F)   (PSD shift, prefolded by PF=1/3 so ||B0||_F ~ 1)
  B <- B^2, renormalized by 1/||B||_F^2 on steps {2,5,8}   (12 steps total)
  B converges to c*P (ground-state projector); loss terms from P via rowsums.

Implementation notes:
  - complex 16x16 embedded as real 32x32 M(B) = [[Br,-Bi],[Bi,Br]]; 4 samples
    stacked per 128 partitions; per-sample squaring = one 32x32 PE-tile matmul
    with 16-col moving operand ([Br;Bi] half of M).
  - state per step: wb [128, 32*NQ] f16 holds full M; left 16 cols/quad are
    the t-form [Br;Bi] (cast directly from PSUM), right 16 cols [-Bi;Br] are
    stream-shuffled from u = t*signp.
  - ||H||_F^2 computed as y^T G34 y (G34 precomputed host-side, 34-dim
    y=(x,1,q)); fro itself feeds back as a 35th contraction row so the H-build
    matmul emits PF*(fro*I - H) directly (no separate diagonal add).
  - 4-slab pipelining keeps PE (the bottleneck) continuously fed; elementwise
    work is split across Act/DVE/Pool so each stays under the PE step time.
"""

import numpy as np

N, D, DIM = 4096, 32, 16
NCORES = 8
NS = N // NCORES          # 512 samples per core
NQ = NS // 4              # 128 quads (4 samples per 128 partitions)
EPS = 1e-5
LAM = 0.1
PF = 1.0 / 3.0            # prefold of B0
KSTEPS = 12
NORM_STEPS = (2, 5, 8)
NSLAB = 4
QS = NQ // NSLAB          # 32 quads per slab
GSH = 8                   # G34 scaled by 2^-GSH to keep f16 products in range

_prog_cache = {}

# ---- cinA packed layout (bytes per partition) -----------------------------
A_XTH = 0                 # f16 [35, 512]   1024B  (row 34 device-written fro)
A_WH = 1024               # f16 [35, 512]   1024B
A_G34 = 2048              # f16 [34, 64]    128B   (34 used, padded)
A_ONES = 2176             # f16 [34, 128]   256B
A_SIGNP = 2432            # f32 [128, 1]    4B (pad 16)
A_MASKB = 2448            # f32 [128, 128]  512B
CA = 2960
# ---- cinB ------------------------------------------------------------------
B_WPOS = 0                # f32 [128, 128]  512B
B_WEA2 = 512              # f32 [128, 128]  512B
B_XBLK = 1024             # f32 [128, 128]  512B
CB = 1536


def _build_host_tensors(A_real, A_imag, X):
    A = (A_real + 1j * A_imag).astype(np.complex64)
    Sc = A + np.conj(np.transpose(A, (0, 2, 1)))        # [D,16,16] Hermitian
    Sr, Si = Sc.real.astype(np.float64), Sc.imag.astype(np.float64)
    G = np.einsum('dij,dkj->ik', A, A.conj())
    Gr, Gi = G.real.astype(np.float64), G.imag.astype(np.float64)
    cA = A.sum(axis=1)                                   # [D,16] colsum over i
    cA2 = (A @ A).sum(axis=1)

    # WH[k, 32j+m]: contraction k: 0 = fro (device-written), 1..32 = d,
    # 33 = const, 34 = q.  Emits pm = PF*(fro*I - H) in t-layout.
    WH = np.zeros((35, 512), np.float64)
    for j in range(DIM):
        c = 32 * j
        WH[0, c+j] = 1.0                                 # fro * I
        WH[1:1+D, c:c+16] = 0.5 * Sr[:, :, j]            # -(-0.5 Sr) = +0.5
        WH[1:1+D, c+16:c+32] = 0.5 * Si[:, :, j]
        WH[33, c:c+16] = -0.5 * Gr[:, j]
        WH[33, c+j] -= EPS
        WH[33, c+16:c+32] = -0.5 * Gi[:, j]
        WH[34, c+j] = -0.5
    WH *= PF

    # G35 for fro^2 = y^T G y (y = rows 1..34 of xth: (x, 1, q)); row/col 0
    # (the fro row) is zero so the runtime fro value never contributes.
    Wraw = WH[1:35].reshape(34, 16, 32) / PF
    G34 = np.einsum('kjm,ljm->kl', Wraw, Wraw) * (2.0 ** -GSH)
    G35 = np.zeros((35, 35))
    G35[1:35, 1:35] = G34

    MASKB = np.zeros((128, 128), np.float32)
    for b in range(4):
        MASKB[32*b:32*b+32, 32*b:32*b+32] = 1.0
    SIGNP = np.ones((128, 1), np.float32)
    for s in range(4):
        SIGNP[32*s+16:32*s+32, 0] = -1.0
    # finish functionals: rs is rowsums of t-form [Pr; +Pi]
    #   pos_raw[32s+d, q] = sum_i cAr[d,i]*rr[i] - cAi[d,i]*ri[i]
    WPOS = np.zeros((128, 128), np.float32)
    WEA2 = np.zeros((128, 128), np.float32)
    for s in range(4):
        b = 32 * s
        WPOS[b:b+16, b:b+32] = cA.real.T
        WPOS[b+16:b+32, b:b+32] = -cA.imag.T
        WEA2[b:b+16, b:b+32] = cA2.real.T
        WEA2[b+16:b+32, b:b+32] = -cA2.imag.T
    ONES34 = np.ones((35, 128), np.float32)

    def put(buf, off, arr, dt):
        a = np.ascontiguousarray(arr.astype(dt))
        b = a.view(np.uint8).reshape(a.shape[0], -1)
        buf[:a.shape[0], off:off+b.shape[1]] = b

    per_core = []
    for c in range(NCORES):
        Xc = np.asarray(X[c*NS:(c+1)*NS], np.float64)    # [512, 32]
        q = (Xc ** 2).sum(1)
        XTH = np.zeros((35, 512), np.float64)
        XBLK = np.zeros((128, 128), np.float32)
        for s in range(4):
            idx = np.arange(NQ) * 4 + s                  # sample (q, s)
            XTH[1:1+D, 128*s:128*s+128] = Xc[idx].T
            XTH[33, 128*s:128*s+128] = 1.0
            XTH[34, 128*s:128*s+128] = q[idx]
            XBLK[32*s:32*s+32, :] = Xc[idx].T.astype(np.float32)
        bufA = np.zeros((128, CA), np.uint8)
        put(bufA, A_XTH, XTH, np.float16)
        put(bufA, A_WH, WH, np.float16)
        g = np.zeros((35, 64), np.float64)
        g[:, :35] = G35
        put(bufA, A_G34, g, np.float16)
        put(bufA, A_ONES, ONES34, np.float16)
        put(bufA, A_SIGNP, SIGNP, np.float32)
        put(bufA, A_MASKB, MASKB, np.float32)
        bufB = np.zeros((128, CB), np.uint8)
        put(bufB, B_WPOS, WPOS, np.float32)
        put(bufB, B_WEA2, WEA2, np.float32)
        put(bufB, B_XBLK, XBLK, np.float32)
        per_core.append({"cina": bufA, "cinb": bufB})
    return per_core


def build_program(ksteps=KSTEPS, norm_steps=NORM_STEPS, debug=False):
    import concourse.bass as bass
    import concourse.bacc as bacc
    import concourse.mybir as mybir
    import concourse.tile as tile
    from contextlib import ExitStack

    f16, f32 = mybir.dt.float16, mybir.dt.float32
    u8, u32 = mybir.dt.uint8, mybir.dt.uint32
    Alu = mybir.AluOpType
    Act = mybir.ActivationFunctionType
    X_AX = mybir.AxisListType.X
    HSWAP = list(range(16, 32)) + list(range(0, 16))
    norm_set = set(norm_steps)

    nc = bacc.Bacc()
    d_cina = nc.dram_tensor("cina", [128, CA], u8, kind="ExternalInput")
    d_cinb = nc.dram_tensor("cinb", [128, CB], u8, kind="ExternalInput")
    d_out = nc.dram_tensor("out", [128, 1], f32, kind="ExternalOutput")
    if debug:
        d_dbg_fro = nc.dram_tensor("dbg_fro", [128, 512], f32,
                                   kind="ExternalOutput")
        d_dbg_invf = nc.dram_tensor("dbg_invf", [128, 128], f32,
                                    kind="ExternalOutput")
        d_dbg_wb0 = nc.dram_tensor("dbg_wb0", [128, 4096], f16,
                                   kind="ExternalOutput")
        d_dbg_wb1 = nc.dram_tensor("dbg_wb1", [128, 4096], f16,
                                   kind="ExternalOutput")
        d_dbg_rs = nc.dram_tensor("dbg_rs", [128, 128], f32,
                                  kind="ExternalOutput")
        d_dbg_prt = nc.dram_tensor("dbg_prt", [128, 128], f32,
                                   kind="ExternalOutput")
        d_dbg_wb2 = nc.dram_tensor("dbg_wb2", [128, 4096], f16,
                                   kind="ExternalOutput")
        d_dbg_wb3 = nc.dram_tensor("dbg_wb3", [128, 4096], f16,
                                   kind="ExternalOutput")
        d_dbg_scl = nc.dram_tensor("dbg_scl", [128, 128], f32,
                                   kind="ExternalOutput")
        d_dbg_wbs = {
            k: nc.dram_tensor(f"dbg_wbs{k}", [128, 4096], f16,
                              kind="ExternalOutput")
            for k in range(3, KSTEPS)
        }

    with tile.TileContext(nc) as tc, ExitStack() as ctx:
        cpool = ctx.enter_context(tc.tile_pool(name="consts", bufs=1))
        spool = ctx.enter_context(tc.tile_pool(name="state", bufs=2))
        wpool = ctx.enter_context(tc.tile_pool(name="work", bufs=2))
        upool = ctx.enter_context(tc.tile_pool(name="uslab", bufs=3))
        qpool = ctx.enter_context(tc.tile_pool(name="small", bufs=2))
        pmpool = ctx.enter_context(tc.tile_pool(name="psum_pm", bufs=4,
                                                space="PSUM"))
        p1ctx = ExitStack()
        p1pool = p1ctx.enter_context(tc.tile_pool(name="psum_p1", bufs=1,
                                                  space="PSUM"))

        cina = cpool.tile([128, CA], u8, tag="cina")
        cinb = cpool.tile([128, CB], u8, tag="cinb")
        nc.sync.dma_start(cina[:, :], d_cina[:, :])
        nc.sync.dma_start(cinb[:, :], d_cinb[:, :])

        xth = cina[:, A_XTH:A_XTH+1024].bitcast(f16)[0:35, :]
        wh = cina[:, A_WH:A_WH+1024].bitcast(f16)[0:35, :]
        g34 = cina[:, A_G34:A_G34+128].bitcast(f16)[0:35, :]
        ones34 = cina[:, A_ONES:A_ONES+256].bitcast(f16)[0:35, :]
        signp = cina[:, A_SIGNP:A_SIGNP+4].bitcast(f32)
        maskb = cina[:, A_MASKB:A_MASKB+512].bitcast(f32)
        wpos = cinb[:, B_WPOS:B_WPOS+512].bitcast(f32)
        wea2 = cinb[:, B_WEA2:B_WEA2+512].bitcast(f32)
        xblk = cinb[:, B_XBLK:B_XBLK+512].bitcast(f32)

        # ---------------- warmup: keep PE busy from t=0 --------------------
        wz = wpool.tile([128, 512], f16, tag="wz")
        nc.gpsimd.memset(wz[:, :], 0.0)
        pwarm = p1pool.tile([128, 512], f32, tag="warm")
        for _ in range(4):
            nc.tensor.matmul(pwarm[:, 0:256], wz[0:32, 0:128], wz[0:32, 0:256],
                             start=True, stop=True)

        # ---------------- fro chain: fro^2 = y^T G34 y ---------------------
        gy = p1pool.tile([35, 512], f32, tag="gy")
        nc.tensor.matmul(gy[:, :], g34[:, 0:35], xth[:, :],
                         start=True, stop=True)
        prod = wpool.tile([35, 512], f16, tag="prod")
        nc.vector.tensor_tensor(prod[:, :], gy[:, :], xth[:, :],
                                op=Alu.mult)
        fro2b = p1pool.tile([128, 512], f32, tag="fro2b")
        nc.tensor.matmul(fro2b[:, :], ones34[:, :], prod[:, :],
                         start=True, stop=True)
        for _ in range(4):
            nc.tensor.matmul(pwarm[:, 0:256], wz[0:32, 0:128], wz[0:32, 0:256],
                             start=True, stop=True)
        # xth row 0 <- fro = sqrt(fro2b * 2^GSH)
        nc.scalar.activation(xth[0:1, :], fro2b[0:1, :], Act.Sqrt,
                             scale=float(2.0 ** GSH))
        # invf[p, q] = 1/fro of sample (q, s(p))
        invsq = wpool.tile([128, 128], f32, tag="invsq")
        for s in range(4):
            nc.vector.reciprocal(invsq[32*s:32*s+32, :],
                                 fro2b[32*s:32*s+32, 128*s:128*s+128])
        invf = wpool.tile([128, 128], f32, tag="invf")
        nc.scalar.activation(invf[:, :], invsq[:, :], Act.Sqrt,
                             scale=float(2.0 ** -GSH))

        # ---------------- helpers -----------------------------------------
        def wb_left(wb_t, sl):
            return wb_t[:, :].rearrange("p (q c) -> p q c", c=32)[
                :, sl*QS:(sl+1)*QS, 0:16]

        def wb_right_u32(wb_t, sl):
            return wb_t[:, :].bitcast(u32).rearrange("p (q w) -> p q w", w=16)[
                :, sl*QS:(sl+1)*QS, 8:16]

        def emit_trio(wbn, pm_t, sl, scl=None, cast_engine="act",
                      last=False):
            """pm [128, 16*QS] -> wbn left (t-form), u, wbn right."""
            dst = wb_left(wbn, sl)
            src = pm_t[:, :].rearrange("p (q j) -> p q j", j=16)
            if scl is not None:
                nc.vector.tensor_tensor(
                    dst, src,
                    scl.unsqueeze(-1).broadcast_to([128, QS, 16]),
                    op=Alu.mult)
            elif cast_engine == "act":
                nc.scalar.activation(dst, src, Act.Copy)
            else:
                nc.vector.tensor_copy(dst, src)
            if last:
                return
            us = upool.tile([128, 16*QS], f16, tag=f"u{sl % 2}")
            if cast_engine == "dve_u_act":
                nc.scalar.activation(us[:, :], wb_left(wbn, sl), Act.Copy,
                                     scale=signp[:, :])
            else:
                nc.vector.tensor_scalar_mul(
                    us[:, :].rearrange("p (q j) -> p q j", j=16),
                    wb_left(wbn, sl), signp[:, :])
            nc.vector.stream_shuffle(
                wb_right_u32(wbn, sl),
                us[:, :].bitcast(u32).rearrange("p (q w) -> p q w", w=8),
                mask=HSWAP)

        def emit_norm_prep(wb_t, sl, sq_t, pr_t, trp_t, inv_t=None):
            """fro^2 of state per sample -> trp_t[:, sl*QS:...] (psum)."""
            sq_ap = sq_t[:, :].rearrange("p (q j) -> p q j", j=16)
            if sl % 2 == 0:
                nc.scalar.activation(sq_ap, wb_left(wb_t, sl), Act.Square)
            else:
                nc.gpsimd.tensor_tensor(sq_ap, wb_left(wb_t, sl),
                                        wb_left(wb_t, sl), op=Alu.mult)
            nc.vector.tensor_reduce(
                pr_t[:, :], sq_ap, axis=X_AX, op=Alu.add)
            nc.tensor.matmul(trp_t[:, sl*QS:(sl+1)*QS], maskb[:, :],
                             pr_t[:, :], start=True, stop=True)
            nc.vector.reciprocal(inv_t[:, sl*QS:(sl+1)*QS],
                                 trp_t[:, sl*QS:(sl+1)*QS])

        # ---------------- phase 1: H build -> B0 ---------------------------
        wb = spool.tile([128, 32*NQ], f16, tag="wb")
        for sl in range(NSLAB):
            ph = pmpool.tile([128, 16*QS], f32, tag="pm")
            for j in range(DIM):
                for s in range(4):
                    nc.tensor.matmul(
                        ph[32*s:32*s+32, 32*j:32*j+32],
                        wh[:, 32*j:32*j+32],
                        xth[:, 128*s+QS*sl:128*s+QS*sl+QS],
                        start=True, stop=True, tile_position=(0, 32*s))
            # cast1 with per-quad 1/fro (DVE), u on Act, shuffle DVE
            dst = wb_left(wb, sl)
            nc.vector.tensor_tensor(
                dst, ph[:, :].rearrange("p (j q) -> p q j", j=16),
                invf[:, QS*sl:QS*(sl+1)].unsqueeze(-1)
                    .broadcast_to([128, QS, 16]),
                op=Alu.mult)
            us = upool.tile([128, 16*QS], f16, tag=f"u{sl % 2}")
            nc.scalar.activation(us[:, :], wb_left(wb, sl), Act.Copy,
                                 scale=signp[:, :])
            nc.vector.stream_shuffle(
                wb_right_u32(wb, sl),
                us[:, :].bitcast(u32).rearrange("p (q w) -> p q w", w=8),
                mask=HSWAP)

        if debug:
            frocp = wpool.tile([128, 512], f32, tag="frocp")
            nc.vector.tensor_copy(frocp[:, :], fro2b[:, :])
            nc.sync.dma_start(d_dbg_fro[:, :], frocp[:, :])
            nc.sync.dma_start(d_dbg_invf[:, :], invf[:, :])
            nc.sync.dma_start(d_dbg_wb0[:, :], wb[:, :])

        # ---------------- iteration ----------------------------------------
        p1ctx.close()
        smpool = ctx.enter_context(tc.tile_pool(name="psum_sm", bufs=2,
                                                space="PSUM"))
        scl_t = None
        sq_t = None
        for k in range(ksteps):
            last = (k == ksteps - 1)
            is_norm = k in norm_set
            prep_next = (k + 1) in norm_set or k == ksteps - 2
            wbn = spool.tile([128, 32*NQ], f16, tag="wb")
            if prep_next:
                trp_next = smpool.tile([128, NQ], f32, tag="trp")
                inv_next = qpool.tile([128, NQ], f32, tag="scl")
            prep_q = []
            for sl in range(NSLAB):
                pm = pmpool.tile([128, 16*QS], f32, tag="pm")
                for qq in range(QS):
                    q = sl * QS + qq
                    for s in range(4):
                        nc.tensor.matmul(
                            pm[32*s:32*s+32, 16*qq:16*qq+16],
                            wb[32*s:32*s+32, 32*q:32*q+32],
                            wb[32*s:32*s+32, 32*q:32*q+16],
                            start=True, stop=True,
                            tile_position=(32*s, 32*s))
                if is_norm:
                    emit_trio(wbn, pm, sl,
                              scl=scl_t[:, sl*QS:(sl+1)*QS],
                              cast_engine="dve_u_act", last=last)
                else:
                    emit_trio(wbn, pm, sl, cast_engine="act", last=last)
                if prep_next:
                    # fro^2 of state_{k+1}; lag preps 2 slabs behind trios
                    prep_q.append(sl)
                    if len(prep_q) > 2:
                        slp = prep_q.pop(0)
                        sq = qpool.tile([128, 16*QS], f32, tag=f"sq{slp % 2}")
                        pr = qpool.tile([128, QS], f32, tag=f"pr{slp % 2}")
                        emit_norm_prep(wbn, slp, sq, pr, trp_next, inv_next)
            for slp in prep_q:
                sq = qpool.tile([128, 16*QS], f32, tag=f"sq{slp % 2}")
                pr = qpool.tile([128, QS], f32, tag=f"pr{slp % 2}")
                emit_norm_prep(wbn, slp, sq, pr, trp_next, inv_next)
            wb = wbn
            if debug and k == 0:
                nc.sync.dma_start(d_dbg_wb1[:, :], wb[:, :])
            if debug and k == 1:
                nc.sync.dma_start(d_dbg_wb2[:, :], wb[:, :])
            if debug and k == 2:
                nc.sync.dma_start(d_dbg_wb3[:, :], wb[:, :])
            if debug and k >= 3:
                nc.sync.dma_start(d_dbg_wbs[k][:, :], wb[:, :])
            if prep_next:
                scl_t = inv_next
                if debug and k == 1:
                    sclcp = wpool.tile([128, NQ], f32, tag="sclcp")
                    nc.vector.tensor_copy(sclcp[:, :], scl_t[:, :])
                    nc.sync.dma_start(d_dbg_scl[:, :], sclcp[:, :])

        # ---------------- finish -------------------------------------------
        # rowsums of t-form state (P up to scale); 1/tr(P) = scl_t from the
        # k=ksteps-2 prep (tr(state_11^2) = ||state_11||_F^2, step 11 unnormed)
        rs = wpool.tile([128, NQ], f32, tag="rs")
        for sl in range(NSLAB):
            nc.vector.tensor_reduce(
                rs[:, sl*QS:(sl+1)*QS],
                wb_left(wb, sl), axis=X_AX, op=Alu.add)
        invt = scl_t

        if debug:
            nc.sync.dma_start(d_dbg_rs[:, :], rs[:, :])
            nc.sync.dma_start(d_dbg_prt[:, :], invt[:, :])

        pos = smpool.tile([128, NQ], f32, tag="fin")
        nc.tensor.matmul(pos[:, :], wpos[:, :], rs[:, :], start=True,
                         stop=True)
        ea2 = smpool.tile([128, NQ], f32, tag="fin")
        nc.tensor.matmul(ea2[:, :], wea2[:, :], rs[:, :], start=True,
                         stop=True)
        posn = wpool.tile([128, NQ], f32, tag="posn")
        nc.vector.tensor_tensor(posn[:, :], pos[:, :], invt[:, :],
                                op=Alu.mult)
        ea2n = wpool.tile([128, NQ], f32, tag="ea2n")
        nc.vector.tensor_tensor(ea2n[:, :], ea2[:, :], invt[:, :],
                                op=Alu.mult)
        terr = wpool.tile([128, NQ], f32, tag="terr")
        nc.vector.tensor_tensor(terr[:, :], posn[:, :], xblk[:, :],
                                op=Alu.subtract)
        t2 = wpool.tile([128, NQ], f32, tag="t2")
        nc.scalar.activation(t2[:, :], terr[:, :], Act.Square)
        p2 = wpool.tile([128, NQ], f32, tag="p2")
        nc.scalar.activation(p2[:, :], posn[:, :], Act.Square)
        vv = wpool.tile([128, NQ], f32, tag="vv")
        nc.vector.tensor_tensor(vv[:, :], ea2n[:, :], p2[:, :],
                                op=Alu.subtract)
        r = wpool.tile([128, NQ], f32, tag="r")
        nc.vector.scalar_tensor_tensor(r[:, :], vv[:, :], LAM, t2[:, :],
                                       op0=Alu.mult, op1=Alu.add)
        outv = wpool.tile([128, 1], f32, tag="outv")
        nc.vector.tensor_reduce(outv[:, :], r[:, :], axis=X_AX, op=Alu.add)
        nc.sync.dma_start(d_out[:, :], outv[:, :])
    nc.compile()
    return nc


def kernel(A_real, A_imag, X):
    from concourse.bass_utils import run_bass_kernel_spmd

    per_core = _build_host_tensors(
        np.asarray(A_real, np.float32), np.asarray(A_imag, np.float32),
        np.asarray(X, np.float32))

    if "nc" not in _prog_cache:
        _prog_cache["nc"] = build_program()
    nc = _prog_cache["nc"]

    res = run_bass_kernel_spmd(nc, per_core, list(range(NCORES)))
    total = 0.0
    for c in range(NCORES):
        total += float(np.asarray(res.results[c]["out"], np.float64).sum())
    return np.float32(total / N)
